# revision 32
# baseline (speedup 1.0000x reference)
"""Trainium2 Bass kernel for nn_MinamoTopoModel (3-layer GAT + mean-pool + FC).

Strategy (8 NeuronCores, SPMD):
  - Nodes partitioned contiguously across cores (2500/core); edges assigned by
    destination core so segment-softmax + scatter-add stay local.
  - Per layer, dense stage: h = x @ W_aug where W_aug = [W | va_s | va_d]
    (va = per-head contraction of W with the attention vectors, computed on
    host) so per-node attention logits es/ed come out of the same matmul.
    Rows go to two DRAM tensors: a tiny tail augS [nloc, 32] (es|ed) and the
    feature augF [nloc, dd] (merged into one 544-wide tensor for layer 3).
  - Exchange: Shared-output AllGathers, ordered S -> phase1 -> F so that
    phase 1 runs while the big feature AllGather is still on the wire.
  - Phase 1 (during feature AG): per block, one batched indirect gather of
    the 32B source tails, per-tile OHT matmul to expand per-dst ed
    (host-precomputed one-hots), batched w = exp(leaky(es+ed)) over all
    tiles (shift-invariant softmax, max-subtraction skipped), den += OH^T @ w.
  - Phase 2 (after feature AG): per tile, gather feature rows, per-head
    weight by w (vector/scalar engines split), out += OH^T @ fw.  Epilogue:
    per-head 1/den scaling on the scalar engine, leaky on vector, PE
    transposes packed 8-per-PSUM-bank, two contiguous DMA writes per block
    into the next layer's [MT, 128, KT*128] lhsT layout.
  - Layer 3 (heads=1) feeds per-block matmuls with graph one-hots ->
    pooled sums [16, 512]; AllReduce; mean, FC, leaky -> out.
"""

import math
import os
from contextlib import ExitStack

import numpy as np
import ml_dtypes

import concourse.bass as bass
import concourse.bacc as bacc
import concourse.mybir as mybir
import concourse.tile as tile
from concourse.bass import IndirectOffsetOnAxis
from concourse.masks import make_identity

BF16 = mybir.dt.bfloat16
F32 = mybir.dt.float32
I32 = mybir.dt.int32
AX = mybir.AxisListType
OP = mybir.AluOpType
ACT_EXP = mybir.ActivationFunctionType.Exp
ACT_COPY = mybir.ActivationFunctionType.Copy

SLOPE = 0.2
G = 16  # graphs
NCORES = 8

# offload half the per-head weighting / scaling to the scalar engine
SCALAR_OFFLOAD = True

bf16np = ml_dtypes.bfloat16


# --------------------------------------------------------------------------
# Host-side preprocessing
# --------------------------------------------------------------------------

def _pack_blocks(deg, sizes, cap):
    """LPT bin-pack local dsts (by degree) into len(sizes) bins; bin b takes
    <= sizes[b] dsts and <= cap edges.  Returns per-bin dst lists or None."""
    order = np.argsort(-deg, kind="stable")
    nb = len(sizes)
    loads = np.zeros(nb, dtype=np.int64)
    cnts = np.zeros(nb, dtype=np.int64)
    bins = [[] for _ in range(nb)]
    for d in order:
        g = deg[d]
        placed = False
        for i in np.argsort(loads, kind="stable"):
            if cnts[i] < sizes[i] and loads[i] + g <= cap:
                loads[i] += g
                cnts[i] += 1
                bins[i].append(d)
                placed = True
                break
        if not placed:
            return None
    return bins


def preprocess(x, edge_index, batch, cfg):
    """Build per-core metadata + permutations + one-hot tiles."""
    N, NLOC, NB = cfg["N"], cfg["NLOC"], cfg["NB"]
    HROWS = cfg["HROWS"]
    sizes = cfg["sizes"]
    ei = np.asarray(edge_index).astype(np.int64)
    bat = np.asarray(batch).astype(np.int64)
    x = np.asarray(x, dtype=np.float32)
    loop = np.arange(N, dtype=np.int64)
    src = np.concatenate([ei[0], loop])
    dst = np.concatenate([ei[1], loop])

    deg = np.bincount(dst, minlength=N)

    avg_cap = deg.reshape(NCORES, NLOC).sum(1).max() / NB
    T = max(1, math.ceil(avg_cap / 128))
    all_bins = None
    while True:
        cap = (T + 2) * 128
        all_bins = []
        ok = True
        for c in range(NCORES):
            bins = _pack_blocks(deg[c * NLOC:(c + 1) * NLOC], sizes, cap)
            if bins is None:
                ok = False
                break
            all_bins.append(bins)
        if ok:
            break
        T += 1
        assert T <= 64, "block packing failed"
    loads = np.zeros((NCORES, NB), dtype=np.int64)
    for c in range(NCORES):
        for b, bl in enumerate(all_bins[c]):
            loads[c, b] = deg[np.asarray(bl, dtype=np.int64) + c * NLOC].sum()
    tiles = np.maximum(1, np.ceil(loads.max(0) / 128).astype(int))
    offs = np.concatenate([[0], np.cumsum(tiles)])
    cfg = dict(cfg)
    cfg["tiles"] = [int(t) for t in tiles]
    cfg["offs"] = [int(o) for o in offs]
    TOT = int(offs[-1])
    cfg["TOT"] = TOT

    # permutation: per core, orig local id -> new local id (b*128 + j)
    perm = np.full((NCORES, NLOC), -1, dtype=np.int64)
    for c in range(NCORES):
        for b, bl in enumerate(all_bins[c]):
            for j, d in enumerate(bl):
                perm[c, d] = b * 128 + j
    assert (perm >= 0).all()
    owner = np.arange(N) // NLOC
    nl_of = perm[owner, np.arange(N) % NLOC]
    # [8, 2, HROWS] core-major single-AllGather layout
    g2r = owner * (2 * HROWS) + nl_of

    ecore = dst // NLOC
    metas, ohs, ohts, gone, xT = [], [], [], [], []
    NROWS = cfg["NROWS"]
    for c in range(NCORES):
        mrow = np.zeros((TOT * 128,), dtype=np.int32)  # gather row id
        ohf = np.zeros((TOT * 128, 128), dtype=bf16np)
        sel = np.nonzero(ecore == c)[0]
        nl = perm[c, dst[sel] - c * NLOC]
        blk = nl // 128
        slot = nl % 128
        rows = g2r[src[sel]]
        order = np.lexsort((rows, blk))
        blk, slot, rows = blk[order], slot[order], rows[order]
        pos = 0
        for b in range(NB):
            cnt = int((blk == b).sum())
            base = offs[b] * 128
            assert cnt <= tiles[b] * 128
            mrow[base:base + cnt] = rows[pos:pos + cnt]
            ohf[np.arange(base, base + cnt), slot[pos:pos + cnt]] = 1.0
            pos += cnt
        # pre-rearranged layouts: [128 partitions, TOT, 128]
        metas.append(np.ascontiguousarray(
            mrow.reshape(TOT, 128).T).astype(np.int32))
        oh3 = ohf.reshape(TOT, 128, 128)
        ohs.append(np.ascontiguousarray(oh3.transpose(1, 0, 2)))
        ohts.append(np.ascontiguousarray(oh3.transpose(2, 0, 1)))

        inv = np.empty(NLOC, dtype=np.int64)
        inv[perm[c]] = np.arange(NLOC)
        orig = inv + c * NLOC
        go = np.zeros((NROWS, G), dtype=bf16np)
        go[np.arange(NLOC), bat[orig]] = 1.0
        gone.append(go)

        xt = np.zeros((x.shape[1], NROWS), dtype=np.float32)
        xt[:, :NLOC] = x[orig].T
        xT.append(xt)

    cnts = np.bincount(bat, minlength=G).astype(np.float32)
    crecip = (1.0 / np.maximum(cnts, 1.0)).reshape(G, 1).astype(np.float32)

    return cfg, metas, ohs, ohts, gone, xT, crecip


def _fold_aug_weights(W, a_s, a_d, heads, ch):
    """W_aug = [W | va_s | va_d] with va[k, h] = sum_c W[k, h*ch+c] * a[h, c],
    each tail padded to 8 columns."""
    W = np.asarray(W, dtype=np.float64)
    a_s = np.asarray(a_s, dtype=np.float64).reshape(heads, ch)
    a_d = np.asarray(a_d, dtype=np.float64).reshape(heads, ch)
    k = W.shape[0]
    W3 = W.reshape(k, heads, ch)
    va_s = np.einsum("khc,hc->kh", W3, a_s)
    va_d = np.einsum("khc,hc->kh", W3, a_d)
    pad = np.zeros((k, 8 - heads), dtype=np.float64)
    Waug = np.concatenate([W, va_s, pad, va_d, pad], axis=1)
    return Waug.astype(np.float32).astype(bf16np)


# --------------------------------------------------------------------------
# Bass program
# --------------------------------------------------------------------------

def build_program(cfg):
    N, NLOC, NB = cfg["N"], cfg["NLOC"], cfg["NB"]
    sizes = cfg["sizes"]
    btiles, offs = cfg["tiles"], cfg["offs"]
    TMAX = max(btiles)
    TOT = cfg["TOT"]
    NROWS = cfg["NROWS"]
    HROWS = cfg["HROWS"]
    MT = NROWS // 128
    TILEF, EMB, HID, OUT, FEAT, HEADS = 32, 128, 256, 512, 512, 8
    D1, D2, D3 = HEADS * HID, HEADS * HID, OUT   # 2048, 2048, 512

    nc = bacc.Bacc(num_devices=NCORES)

    # ---------------- I/O ----------------
    xT_in = nc.dram_tensor("xT_in", [TILEF, NROWS], F32, kind="ExternalInput")
    meta_in = nc.dram_tensor("meta", [128, TOT], I32, kind="ExternalInput")
    oh_in = nc.dram_tensor("oh", [128, TOT, 128], BF16, kind="ExternalInput")
    oht_in = nc.dram_tensor("oht", [128, TOT, 128], BF16, kind="ExternalInput")
    gone_in = nc.dram_tensor("gone", [NROWS, G], BF16, kind="ExternalInput")
    crecip_in = nc.dram_tensor("crecip", [G, 1], F32, kind="ExternalInput")
    W0_in = nc.dram_tensor("W0", [TILEF, EMB], F32, kind="ExternalInput")
    b0_in = nc.dram_tensor("b0", [1, EMB], BF16, kind="ExternalInput")
    Ws_in = {}
    for nm, kdim, ndim in (("W1", EMB, D1 + 16), ("W2", D1, D2 + 16),
                           ("W3", D2, D3 + 16), ("Wf", OUT, FEAT)):
        Ws_in[nm] = nc.dram_tensor(nm, [kdim, ndim], BF16, kind="ExternalInput")
    bf_in = nc.dram_tensor("bfc", [1, FEAT], F32, kind="ExternalInput")
    out_ext = nc.dram_tensor("out", [G, FEAT], F32, kind="ExternalOutput")

    with tile.TileContext(nc) as tc, ExitStack() as ctx:
        dram = ctx.enter_context(tc.tile_pool(name="dram", bufs=1, space="DRAM"))
        cpool = ctx.enter_context(tc.tile_pool(name="consts", bufs=1))
        wpool = ctx.enter_context(tc.tile_pool(name="weights", bufs=1))
        sb = ctx.enter_context(tc.tile_pool(name="work", bufs=2))
        gsp = ctx.enter_context(tc.tile_pool(name="gsp", bufs=3))
        gfp = ctx.enter_context(tc.tile_pool(name="gfp", bufs=4))
        slabp = ctx.enter_context(tc.tile_pool(name="slabp", bufs=3))
        pp_dense = ctx.enter_context(
            tc.tile_pool(name="pp_dense", bufs=1, space="PSUM"))
        pp_attn = ctx.enter_context(
            tc.tile_pool(name="pp_attn", bufs=1, space="PSUM"))
        pp_den = ctx.enter_context(
            tc.tile_pool(name="pp_den", bufs=1, space="PSUM"))
        pp_t = ctx.enter_context(tc.tile_pool(name="pp_t", bufs=2, space="PSUM"))

        # ------------- internal DRAM -------------
        # transposed activations: [MT, 128, KT*128] (contiguous lhsT rows)
        xTb = {0: dram.tile([MT, 128, EMB], BF16, name="xT0"),
               1: dram.tile([MT, 128, D1], BF16, name="xT1"),
               2: dram.tile([MT, 128, D2], BF16, name="xT2")}
        AUW = {1: D1 + 32, 2: D2 + 32, 3: D3 + 32}
        augF_l = {li: dram.tile([2, HROWS, AUW[li]], BF16,
                                name=f"augF{li}l") for li in (1, 2, 3)}
        augF_f = {li: dram.tile([NCORES, 2, HROWS, AUW[li]], BF16,
                                name=f"augF{li}f", addr_space="Shared")
                  for li in (1, 2, 3)}
        pool_in = dram.tile([G, FEAT], F32, name="pool_in")
        pool_out = dram.tile([G, FEAT], F32, name="pool_out",
                             addr_space="Shared")

        # ------------- constants -------------
        ident = cpool.tile([128, 128], BF16, name="ident")
        make_identity(nc, ident[:])

        W0_sb = wpool.tile([TILEF, EMB], F32, name="W0sb")
        nc.sync.dma_start(W0_sb[:], W0_in[:, :])
        b0_sb = wpool.tile([128, EMB], BF16, name="b0sb")
        nc.sync.dma_start(b0_sb[:], b0_in[:, :].to_broadcast([128, EMB]))
        bf_sb = wpool.tile([G, FEAT], F32, name="bfsb")
        nc.sync.dma_start(bf_sb[:], bf_in[:, :].to_broadcast([G, FEAT]))
        crecip_sb = wpool.tile([G, 1], F32, name="crecipsb")
        nc.sync.dma_start(crecip_sb[:], crecip_in[:, :])
        gone_sb = wpool.tile([128, MT, G], BF16, name="gonesb")
        nc.sync.dma_start(
            gone_sb[:], gone_in[:, :].rearrange("(b p) g -> p b g", p=128))
        meta_sb = cpool.tile([128, TOT], I32, name="metasb")
        nc.sync.dma_start(meta_sb[:], meta_in[:, :])
        oh_sb = cpool.tile([128, TOT, 128], BF16, name="ohsb")
        nc.sync.dma_start(oh_sb[:], oh_in[:, :, :])

        def load_layer_weights(nm, kdim, ndim):
            ts_ = []
            for k in range(kdim // 128):
                t = wpool.tile([128, ndim], BF16, name=f"{nm}k{k}",
                               tag=f"Wk{k}", bufs=1)
                nc.sync.dma_start(t[:], Ws_in[nm][k * 128:(k + 1) * 128, :])
                ts_.append(t)
            return ts_

        def leaky(out_ap, in_ap, tmp_tile):
            nc.vector.tensor_scalar_mul(tmp_tile, in_ap, SLOPE)
            nc.vector.tensor_tensor(out=out_ap, in0=in_ap, in1=tmp_tile,
                                    op=OP.max)

        def transpose_out(xn, dd, dst_row):
            """PE-transpose xn [128, dd], packing 8 k-tiles per PSUM bank,
            then one copy + one DMA per pack into dst_row [128, dd]."""
            KTo = dd // 128
            for j0 in range(0, KTo, 8):
                npk = min(8, KTo - j0)
                xtp = pp_t.tile([128, 8, 128], BF16, name="xtp", tag="tps")
                for j in range(npk):
                    k = j0 + j
                    nc.tensor.transpose(xtp[:, j, :],
                                        xn[:, k * 128:(k + 1) * 128], ident[:])
                xts = sb.tile([128, 8, 128], BF16, name="xts", tag="xts",
                              bufs=2)
                nc.vector.tensor_copy(xts[:, 0:npk, :], xtp[:, 0:npk, :])
                nc.sync.dma_start(
                    dst_row[:, j0 * 128:(j0 + npk) * 128],
                    xts[:, 0:npk, :].rearrange("p a b -> p (a b)"))

        # =========== layer 0: x0 = leaky(x @ W0 + b0), stored transposed ====
        for m in range(MT):
            ms = slice(m * 128, (m + 1) * 128)
            lhs0 = sb.tile([TILEF, 128], F32, name="lhs0", tag="lhs0", bufs=2)
            nc.sync.dma_start(lhs0[:], xT_in[:, ms])
            ps = pp_dense.tile([128, 512], F32, name="ps0", tag="dps")
            nc.tensor.matmul(ps[:, :EMB], lhsT=lhs0[:], rhs=W0_sb[:],
                             start=True, stop=True)
            hraw = sb.tile([128, EMB], BF16, name="hraw0", tag="x0t", bufs=2)
            nc.vector.tensor_tensor(out=hraw[:], in0=ps[:, :EMB],
                                    in1=b0_sb[:], op=OP.add)
            tmp = sb.tile([128, EMB], BF16, name="tmp0", tag="x0tmp", bufs=2)
            xo = sb.tile([128, EMB], BF16, name="xo0", tag="x0o", bufs=2)
            leaky(xo[:], hraw[:], tmp[:])
            transpose_out(xo[:], EMB, xTb[0][m])

        # =========== GAT layers ===========
        def gat_layer(li, d_in, dd, heads, xT_prev, xT_out):
            ch = dd // heads
            KT = d_in // 128
            NCH = dd // 512
            W = load_layer_weights(f"W{li}", d_in, dd + 16)

            # ---- dense stage ----
            for m in range(MT):
                rows = min(128, NLOC - m * 128)
                if rows <= 0:
                    continue
                half = m * 128 // HROWS
                r0 = m * 128 - half * HROWS
                lhs_all = sb.tile([128, d_in], BF16, name=f"lhsT{li}",
                                  tag="lhsT", bufs=2)
                nc.sync.dma_start(lhs_all[:], xT_prev[m])
                h_sb = sb.tile([128, dd + 16], BF16, name=f"h{li}",
                               tag="h_sb", bufs=2)
                for nchunk in range(NCH):
                    nsl = slice(nchunk * 512, (nchunk + 1) * 512)
                    ps = pp_dense.tile([128, 512], F32, name=f"dps{li}",
                                       tag="dps")
                    for k in range(KT):
                        nc.tensor.matmul(
                            ps[:], lhsT=lhs_all[:, k * 128:(k + 1) * 128],
                            rhs=W[k][:, nsl],
                            start=(k == 0), stop=(k == KT - 1))
                    nc.vector.tensor_copy(h_sb[:, nsl], ps[:])
                tail_ps = pp_t.tile([128, 16], F32, name=f"tlps{li}",
                                    tag="tps")
                for k in range(KT):
                    nc.tensor.matmul(
                        tail_ps[:], lhsT=lhs_all[:, k * 128:(k + 1) * 128],
                        rhs=W[k][:, dd:dd + 16],
                        start=(k == 0), stop=(k == KT - 1))
                nc.vector.tensor_copy(h_sb[:, dd:dd + 16], tail_ps[:])
                nc.sync.dma_start(augF_l[li][half, r0:r0 + rows, 0:dd + 16],
                                  h_sb[:rows, :])

            nc.gpsimd.collective_compute(
                "AllGather", OP.bypass,
                replica_groups=[list(range(NCORES))],
                ins=[augF_l[li][:, :, :].opt()],
                outs=[augF_f[li][:, :, :, :].opt()])
            aug_rows = augF_f[li][:, :, :, :].rearrange("c a r w -> (c a r) w")

            w_allf = sb.tile([128, TOT, 8], F32, name=f"wallf{li}",
                             tag="wallf", bufs=2)
            pool_ps = None
            if li == 3:
                pool_ps = pp_dense.tile([G, 512], F32, name="poolps",
                                        tag="dps")

            for b in range(NB):
                rows = sizes[b]
                r0 = b * 128
                half = r0 // HROWS
                hr0 = r0 - half * HROWS
                o = offs[b]
                TB = btiles[b]
                # ---- pass A: tail gathers -> w, den, 1/den ----
                ed_blk = sb.tile([128, 8], BF16, name=f"edblk{li}",
                                 tag="edblk", bufs=3)
                if rows < 128:
                    nc.vector.memset(ed_blk[:], 0.0)
                nc.sync.dma_start(
                    ed_blk[:rows, :],
                    augF_l[li][half, hr0:hr0 + rows, dd + 8:dd + 16])
                oht_sl = slabp.tile([128, TMAX, 128], BF16, name=f"oht{li}",
                                    tag="ohtsl")
                nc.sync.dma_start(oht_sl[:, 0:TB, :],
                                  oht_in[:, o:o + TB, :])
                edpe_ps = pp_t.tile([128, TMAX, 8], F32, name=f"edpe{li}",
                                    tag="tps")
                den_ps = pp_den.tile([128, 8], F32, name=f"den{li}", tag="den")
                w_all = sb.tile([128, TMAX, 8], BF16, name=f"wall{li}",
                                tag="wall", bufs=2)
                for t in range(TB):
                    ti = o + t
                    gs = gsp.tile([128, 32], BF16, name=f"gs{li}", tag="gs",
                                  bufs=4)
                    nc.gpsimd.indirect_dma_start(
                        out=gs[:], out_offset=None,
                        in_=aug_rows, element_offset=dd,
                        in_offset=IndirectOffsetOnAxis(
                            ap=meta_sb[:, ti:ti + 1], axis=0))
                    nc.tensor.matmul(edpe_ps[:, t, :], lhsT=oht_sl[:, t, :],
                                     rhs=ed_blk[:, 0:8], start=True, stop=True)
                    e_t = sb.tile([128, 8], F32, name=f"et{li}", tag="et",
                                  bufs=3)
                    nc.vector.tensor_tensor(out=e_t[:], in0=gs[:, 0:8],
                                            in1=edpe_ps[:, t, :], op=OP.add)
                    w1 = sb.tile([128, 8], F32, name=f"w1{li}", tag="w1",
                                 bufs=3)
                    w2 = sb.tile([128, 8], F32, name=f"w2{li}", tag="w2",
                                 bufs=3)
                    nc.scalar.activation(w1[:], e_t[:], ACT_EXP)
                    nc.scalar.activation(w2[:], e_t[:], ACT_EXP, scale=SLOPE)
                    nc.vector.tensor_tensor(out=w_allf[:, ti, :], in0=w1[:],
                                            in1=w2[:], op=OP.max)
                    nc.vector.tensor_copy(w_all[:, t, :], w_allf[:, ti, :])
                    nc.tensor.matmul(den_ps[:], lhsT=oh_sb[:, ti, :],
                                     rhs=w_all[:, t, :],
                                     start=(t == 0), stop=(t == TB - 1))
                den_sb = sb.tile([128, 8], F32, name=f"densb{li}",
                                 tag="densb", bufs=2)
                nc.vector.tensor_scalar_add(den_sb[:], den_ps[:], 1e-16)
                rd = sb.tile([128, 8], F32, name=f"rd{li}", tag="rd", bufs=2)
                nc.vector.reciprocal(rd[:], den_sb[:])

                # ---- pass B: feature gathers + weighted scatter-add ----
                out_ps = pp_attn.tile([128, dd], F32, name=f"oat{li}",
                                      tag="oat")
                for t in range(TB):
                    ti = o + t
                    gf = gfp.tile([128, dd], BF16, name=f"gf{li}", tag="gf",
                                  bufs=3)
                    nc.gpsimd.indirect_dma_start(
                        out=gf[:], out_offset=None,
                        in_=aug_rows,
                        in_offset=IndirectOffsetOnAxis(
                            ap=meta_sb[:, ti:ti + 1], axis=0))
                    fw = sb.tile([128, dd], BF16, name=f"fw{li}", tag="fw",
                                 bufs=2)
                    for h in range(heads):
                        hs = slice(h * ch, (h + 1) * ch)
                        if SCALAR_OFFLOAD and h % 2 == 1:
                            nc.scalar.activation(
                                fw[:, hs], gf[:, hs], ACT_COPY,
                                scale=w_allf[:, ti, h:h + 1])
                        else:
                            nc.vector.tensor_scalar_mul(
                                fw[:, hs], gf[:, hs], w_allf[:, ti, h:h + 1])
                    for nchunk in range(NCH):
                        nsl = slice(nchunk * 512, (nchunk + 1) * 512)
                        nc.tensor.matmul(out_ps[:, nsl],
                                         lhsT=oh_sb[:, ti, :],
                                         rhs=fw[:, nsl],
                                         start=(t == 0), stop=(t == TB - 1))
                # epilogue: xn = leaky(out * 1/den) per head (biases are 0)
                xc = sb.tile([128, dd], BF16, name=f"xc{li}", tag="xc",
                             bufs=2)
                for h in range(heads):
                    hs = slice(h * ch, (h + 1) * ch)
                    if SCALAR_OFFLOAD and h % 2 == 1:
                        nc.scalar.activation(xc[:, hs], out_ps[:, hs],
                                             ACT_COPY, scale=rd[:, h:h + 1])
                    else:
                        nc.vector.tensor_scalar_mul(xc[:, hs], out_ps[:, hs],
                                                    rd[:, h:h + 1])
                xt_ = sb.tile([128, dd], BF16, name=f"xt{li}", tag="xtm",
                              bufs=1)
                xn = sb.tile([128, dd], BF16, name=f"xn{li}", tag="xn",
                             bufs=2)
                leaky(xn[:], xc[:], xt_[:])
                if li < 3:
                    transpose_out(xn[:], dd, xT_out[b])
                else:
                    nc.tensor.matmul(pool_ps[:], lhsT=gone_sb[:, b, :],
                                     rhs=xn[:, :FEAT], start=(b == 0),
                                     stop=(b == NB - 1))
                    if b == NB - 1:
                        psum_sb = sb.tile([G, FEAT], F32, name="psum_sb",
                                          tag="fc16", bufs=3)
                        nc.vector.tensor_copy(psum_sb[:], pool_ps[:])
                        nc.sync.dma_start(pool_in[:, :], psum_sb[:])

        gat_layer(1, EMB, D1, HEADS, xTb[0], xTb[1])
        gat_layer(2, D1, D2, HEADS, xTb[1], xTb[2])
        gat_layer(3, D2, D3, 1, xTb[2], None)

        # =========== pooling reduce + FC ===========
        nc.gpsimd.collective_compute(
            "AllReduce", OP.add, replica_groups=[list(range(NCORES))],
            ins=[pool_in[:, :].opt()], outs=[pool_out[:, :].opt()])
        psum_all = sb.tile([G, FEAT], F32, name="psum_all", tag="fc16", bufs=3)
        nc.sync.dma_start(psum_all[:], pool_out[:, :])
        mean_f = sb.tile([G, FEAT], F32, name="mean_f", tag="fc16", bufs=3)
        nc.vector.tensor_scalar_mul(mean_f[:], psum_all[:], crecip_sb[:, 0:1])
        mean_bf = sb.tile([G, FEAT], BF16, name="mean_bf", tag="fc16", bufs=3)
        nc.vector.tensor_copy(mean_bf[:], mean_f[:])
        Wf_sb = load_layer_weights("Wf", OUT, FEAT)
        fc_ps = pp_dense.tile([G, 512], F32, name="fcps", tag="dps")
        for k in range(OUT // 128):
            mT_ps = pp_t.tile([128, G], BF16, name="mTps", tag="tps")
            nc.tensor.transpose(mT_ps[:], mean_bf[:, k * 128:(k + 1) * 128],
                                ident[:G, :G])
            mT = sb.tile([128, G], BF16, name="mT", tag="mT", bufs=2)
            nc.vector.tensor_copy(mT[:], mT_ps[:])
            nc.tensor.matmul(fc_ps[:], lhsT=mT[:], rhs=Wf_sb[k][:],
                             start=(k == 0), stop=(k == OUT // 128 - 1))
        fc_raw = sb.tile([G, FEAT], F32, name="fc_raw", tag="fc16", bufs=3)
        nc.vector.tensor_tensor(out=fc_raw[:], in0=fc_ps[:], in1=bf_sb[:],
                                op=OP.add)
        fc_t = sb.tile([G, FEAT], F32, name="fc_t", tag="fc16", bufs=3)
        fc_o = sb.tile([G, FEAT], F32, name="fc_o", tag="fc16", bufs=3)
        leaky(fc_o[:], fc_raw[:], fc_t[:])
        nc.sync.dma_start(out_ext[:, :], fc_o[:])

    nc.finalize()
    return nc


# --------------------------------------------------------------------------
# Entry point
# --------------------------------------------------------------------------

def make_cfg(N):
    NLOC = N // NCORES
    NB = (NLOC + 127) // 128
    sizes = [128] * (NLOC // 128) + ([NLOC % 128] if NLOC % 128 else [])
    NROWS = NB * 128
    HROWS = (NB // 2) * 128
    return {"N": N, "NLOC": NLOC, "NB": NB, "sizes": sizes, "NROWS": NROWS,
            "HROWS": HROWS}


def prepare_in_maps(inputs, cfg=None):
    x = np.asarray(inputs["x"], dtype=np.float32)
    N = x.shape[0]
    if cfg is None:
        cfg = make_cfg(N)
    cfg, metas, ohs, ohts, gone, xT, crecip = preprocess(
        x, inputs["edge_index"], inputs["batch"], cfg)

    def b16(a):
        return np.asarray(a, dtype=np.float32).astype(bf16np)

    # biases are all zero in this model; assert so the kernel can skip them
    for bn in ("b1", "b2", "b3"):
        assert np.abs(np.asarray(inputs[bn])).max() == 0.0

    shared = {
        "W0": np.asarray(inputs["W0"], np.float32),
        "b0": b16(inputs["b0"]).reshape(1, -1),
        "W1": _fold_aug_weights(inputs["W1"], inputs["a1s"], inputs["a1d"],
                                8, 256),
        "W2": _fold_aug_weights(inputs["W2"], inputs["a2s"], inputs["a2d"],
                                8, 256),
        "W3": _fold_aug_weights(inputs["W3"], inputs["a3s"], inputs["a3d"],
                                1, 512),
        "Wf": b16(inputs["Wf"]),
        "bfc": np.asarray(inputs["bf"], np.float32).reshape(1, -1),
        "crecip": crecip,
    }
    in_maps = []
    for c in range(NCORES):
        m = dict(shared)
        m["xT_in"] = xT[c]
        m["meta"] = metas[c]
        m["oh"] = ohs[c]
        m["oht"] = ohts[c]
        m["gone"] = gone[c]
        in_maps.append(m)
    return cfg, in_maps


_CACHE = {}


def _ensure_ntff_hook():
    """Register the axon NTFF profiling hook if the antenv shim is missing."""
    import sys
    import types
    try:
        from antenv.axon_hooks import get_axon_ntff_profile_hook  # noqa: F401
        return
    except ImportError:
        pass
    try:
        import antenv
        from trn_agent_boot.trn_boot import _ntff_profile_via_ctypes
    except ImportError:
        return
    mod = types.ModuleType("antenv.axon_hooks")
    mod._hook = None
    mod.set_axon_ntff_profile_hook = lambda h: setattr(mod, "_hook", h)
    mod.get_axon_ntff_profile_hook = lambda: mod._hook
    sys.modules["antenv.axon_hooks"] = mod
    antenv.axon_hooks = mod
    try:
        mod._hook = _ntff_profile_via_ctypes("/opt/axon/libaxon_pjrt.so")
    except Exception:
        mod._hook = None


def kernel(**inputs) -> np.ndarray:
    from concourse.bass_utils import run_bass_kernel_spmd
    if os.environ.get("GNN_TRACE"):
        _ensure_ntff_hook()
    cfg, in_maps = prepare_in_maps(inputs)
    key = (cfg["N"], cfg["NB"], tuple(cfg["tiles"]))
    if key not in _CACHE:
        _CACHE[key] = build_program(cfg)
    nc = _CACHE[key]
    res = run_bass_kernel_spmd(nc, in_maps, core_ids=list(range(NCORES)),
                               trace=bool(os.environ.get("GNN_TRACE")))
    out = res.results[0]["out"]
    kernel.last_exec_time_ns = res.exec_time_ns
    kernel.last_results = res
    return np.asarray(out, dtype=np.float32)


# revision 35
# speedup vs baseline: 1.1181x; 1.1181x over previous
"""Trainium2 Bass kernel for nn_MinamoTopoModel (3-layer GAT + mean-pool + FC).

Strategy (8 NeuronCores, SPMD):
  - Nodes partitioned contiguously across cores (2500/core); edges assigned by
    destination core so segment-softmax + scatter-add stay local.
  - Per layer, dense stage: h = x @ W_aug where W_aug = [W | va_s | va_d]
    (va = per-head contraction of W with the attention vectors, computed on
    host) so per-node attention logits es/ed come out of the same matmul.
    Rows go to two DRAM tensors: a tiny tail augS [nloc, 32] (es|ed) and the
    feature augF [nloc, dd] (merged into one 544-wide tensor for layer 3).
  - Exchange: Shared-output AllGathers, ordered S -> phase1 -> F so that
    phase 1 runs while the big feature AllGather is still on the wire.
  - Phase 1 (during feature AG): per block, one batched indirect gather of
    the 32B source tails, per-tile OHT matmul to expand per-dst ed
    (host-precomputed one-hots), batched w = exp(leaky(es+ed)) over all
    tiles (shift-invariant softmax, max-subtraction skipped), den += OH^T @ w.
  - Phase 2 (after feature AG): per tile, gather feature rows, per-head
    weight by w (vector/scalar engines split), out += OH^T @ fw.  Epilogue:
    per-head 1/den scaling on the scalar engine, leaky on vector, PE
    transposes packed 8-per-PSUM-bank, two contiguous DMA writes per block
    into the next layer's [MT, 128, KT*128] lhsT layout.
  - Layer 3 (heads=1) feeds per-block matmuls with graph one-hots ->
    pooled sums [16, 512]; AllReduce; mean, FC, leaky -> out.
"""

import math
import os
from contextlib import ExitStack

import numpy as np
import ml_dtypes

import concourse.bass as bass
import concourse.bacc as bacc
import concourse.mybir as mybir
import concourse.tile as tile
from concourse.bass import IndirectOffsetOnAxis
from concourse.masks import make_identity

BF16 = mybir.dt.bfloat16
F32 = mybir.dt.float32
I32 = mybir.dt.int32
AX = mybir.AxisListType
OP = mybir.AluOpType
ACT_EXP = mybir.ActivationFunctionType.Exp
ACT_COPY = mybir.ActivationFunctionType.Copy

SLOPE = 0.2
G = 16  # graphs
NCORES = 8

# offload half the per-head weighting / scaling to the scalar engine
SCALAR_OFFLOAD = False

bf16np = ml_dtypes.bfloat16


# --------------------------------------------------------------------------
# Host-side preprocessing
# --------------------------------------------------------------------------

def _pack_blocks(deg, sizes, cap):
    """LPT bin-pack local dsts (by degree) into len(sizes) bins; bin b takes
    <= sizes[b] dsts and <= cap edges.  Returns per-bin dst lists or None."""
    order = np.argsort(-deg, kind="stable")
    nb = len(sizes)
    loads = np.zeros(nb, dtype=np.int64)
    cnts = np.zeros(nb, dtype=np.int64)
    bins = [[] for _ in range(nb)]
    for d in order:
        g = deg[d]
        placed = False
        for i in np.argsort(loads, kind="stable"):
            if cnts[i] < sizes[i] and loads[i] + g <= cap:
                loads[i] += g
                cnts[i] += 1
                bins[i].append(d)
                placed = True
                break
        if not placed:
            return None
    return bins


def preprocess(x, edge_index, batch, cfg):
    """Build per-core metadata + permutations + one-hot tiles."""
    N, NLOC, NB = cfg["N"], cfg["NLOC"], cfg["NB"]
    HROWS = cfg["HROWS"]
    sizes = cfg["sizes"]
    ei = np.asarray(edge_index).astype(np.int64)
    bat = np.asarray(batch).astype(np.int64)
    x = np.asarray(x, dtype=np.float32)
    loop = np.arange(N, dtype=np.int64)
    src = np.concatenate([ei[0], loop])
    dst = np.concatenate([ei[1], loop])

    deg = np.bincount(dst, minlength=N)

    avg_cap = deg.reshape(NCORES, NLOC).sum(1).max() / NB
    T = max(1, math.ceil(avg_cap / 128))
    all_bins = None
    while True:
        cap = (T + 2) * 128
        all_bins = []
        ok = True
        for c in range(NCORES):
            bins = _pack_blocks(deg[c * NLOC:(c + 1) * NLOC], sizes, cap)
            if bins is None:
                ok = False
                break
            all_bins.append(bins)
        if ok:
            break
        T += 1
        assert T <= 64, "block packing failed"
    loads = np.zeros((NCORES, NB), dtype=np.int64)
    for c in range(NCORES):
        for b, bl in enumerate(all_bins[c]):
            loads[c, b] = deg[np.asarray(bl, dtype=np.int64) + c * NLOC].sum()
    tiles = np.maximum(1, np.ceil(loads.max(0) / 128).astype(int))
    offs = np.concatenate([[0], np.cumsum(tiles)])
    cfg = dict(cfg)
    cfg["tiles"] = [int(t) for t in tiles]
    cfg["offs"] = [int(o) for o in offs]
    TOT = int(offs[-1])
    cfg["TOT"] = TOT

    # permutation: per core, orig local id -> new local id (b*128 + j)
    perm = np.full((NCORES, NLOC), -1, dtype=np.int64)
    for c in range(NCORES):
        for b, bl in enumerate(all_bins[c]):
            for j, d in enumerate(bl):
                perm[c, d] = b * 128 + j
    assert (perm >= 0).all()
    owner = np.arange(N) // NLOC
    nl_of = perm[owner, np.arange(N) % NLOC]
    # [8, 2, HROWS] core-major single-AllGather layout
    g2r = owner * (2 * HROWS) + nl_of

    ecore = dst // NLOC
    metas, ohs, ohts, gone, xT = [], [], [], [], []
    NROWS = cfg["NROWS"]
    for c in range(NCORES):
        mrow = np.zeros((TOT * 128,), dtype=np.int32)  # gather row id
        ohf = np.zeros((TOT * 128, 128), dtype=bf16np)
        sel = np.nonzero(ecore == c)[0]
        nl = perm[c, dst[sel] - c * NLOC]
        blk = nl // 128
        slot = nl % 128
        rows = g2r[src[sel]]
        order = np.lexsort((rows, blk))
        blk, slot, rows = blk[order], slot[order], rows[order]
        pos = 0
        for b in range(NB):
            cnt = int((blk == b).sum())
            base = offs[b] * 128
            assert cnt <= tiles[b] * 128
            mrow[base:base + cnt] = rows[pos:pos + cnt]
            ohf[np.arange(base, base + cnt), slot[pos:pos + cnt]] = 1.0
            pos += cnt
        # pre-rearranged layouts: [128 partitions, TOT, 128]
        metas.append(np.ascontiguousarray(
            mrow.reshape(TOT, 128).T).astype(np.int32))
        oh3 = ohf.reshape(TOT, 128, 128)
        ohs.append(np.ascontiguousarray(oh3.transpose(1, 0, 2)))
        ohts.append(np.ascontiguousarray(oh3.transpose(2, 0, 1)))

        inv = np.empty(NLOC, dtype=np.int64)
        inv[perm[c]] = np.arange(NLOC)
        orig = inv + c * NLOC
        go = np.zeros((NROWS, G), dtype=bf16np)
        go[np.arange(NLOC), bat[orig]] = 1.0
        gone.append(go)

        xt = np.zeros((x.shape[1], NROWS), dtype=np.float32)
        xt[:, :NLOC] = x[orig].T
        xT.append(xt)

    cnts = np.bincount(bat, minlength=G).astype(np.float32)
    crecip = (1.0 / np.maximum(cnts, 1.0)).reshape(G, 1).astype(np.float32)

    return cfg, metas, ohs, ohts, gone, xT, crecip


def _fold_aug_weights(W, a_s, a_d, heads, ch):
    """W_aug = [W | va_s | va_d] with va[k, h] = sum_c W[k, h*ch+c] * a[h, c],
    each tail padded to 8 columns."""
    W = np.asarray(W, dtype=np.float64)
    a_s = np.asarray(a_s, dtype=np.float64).reshape(heads, ch)
    a_d = np.asarray(a_d, dtype=np.float64).reshape(heads, ch)
    k = W.shape[0]
    W3 = W.reshape(k, heads, ch)
    va_s = np.einsum("khc,hc->kh", W3, a_s)
    va_d = np.einsum("khc,hc->kh", W3, a_d)
    pad = np.zeros((k, 8 - heads), dtype=np.float64)
    Waug = np.concatenate([W, va_s, pad, va_d, pad], axis=1)
    return Waug.astype(np.float32).astype(bf16np)


# --------------------------------------------------------------------------
# Bass program
# --------------------------------------------------------------------------

def build_program(cfg):
    N, NLOC, NB = cfg["N"], cfg["NLOC"], cfg["NB"]
    sizes = cfg["sizes"]
    btiles, offs = cfg["tiles"], cfg["offs"]
    TMAX = max(btiles)
    TOT = cfg["TOT"]
    NROWS = cfg["NROWS"]
    HROWS = cfg["HROWS"]
    MT = NROWS // 128
    TILEF, EMB, HID, OUT, FEAT, HEADS = 32, 128, 256, 512, 512, 8
    D1, D2, D3 = HEADS * HID, HEADS * HID, OUT   # 2048, 2048, 512

    nc = bacc.Bacc(num_devices=NCORES)

    # ---------------- I/O ----------------
    xT_in = nc.dram_tensor("xT_in", [TILEF, NROWS], F32, kind="ExternalInput")
    meta_in = nc.dram_tensor("meta", [128, TOT], I32, kind="ExternalInput")
    oh_in = nc.dram_tensor("oh", [128, TOT, 128], BF16, kind="ExternalInput")
    oht_in = nc.dram_tensor("oht", [128, TOT, 128], BF16, kind="ExternalInput")
    gone_in = nc.dram_tensor("gone", [NROWS, G], BF16, kind="ExternalInput")
    crecip_in = nc.dram_tensor("crecip", [G, 1], F32, kind="ExternalInput")
    W0_in = nc.dram_tensor("W0", [TILEF, EMB], F32, kind="ExternalInput")
    b0_in = nc.dram_tensor("b0", [1, EMB], BF16, kind="ExternalInput")
    Ws_in = {}
    for nm, kdim, ndim in (("W1", EMB, D1 + 16), ("W2", D1, D2 + 16),
                           ("W3", D2, D3 + 16), ("Wf", OUT, FEAT)):
        Ws_in[nm] = nc.dram_tensor(nm, [kdim, ndim], BF16, kind="ExternalInput")
    bf_in = nc.dram_tensor("bfc", [1, FEAT], F32, kind="ExternalInput")
    out_ext = nc.dram_tensor("out", [G, FEAT], F32, kind="ExternalOutput")

    with tile.TileContext(nc) as tc, ExitStack() as ctx:
        dram = ctx.enter_context(tc.tile_pool(name="dram", bufs=1, space="DRAM"))
        cpool = ctx.enter_context(tc.tile_pool(name="consts", bufs=1))
        wpool = ctx.enter_context(tc.tile_pool(name="weights", bufs=1))
        sb = ctx.enter_context(tc.tile_pool(name="work", bufs=2))
        gsp = ctx.enter_context(tc.tile_pool(name="gsp", bufs=3))
        gfp = ctx.enter_context(tc.tile_pool(name="gfp", bufs=4))
        slabp = ctx.enter_context(tc.tile_pool(name="slabp", bufs=3))
        pp_dense = ctx.enter_context(
            tc.tile_pool(name="pp_dense", bufs=1, space="PSUM"))
        pp_attn = ctx.enter_context(
            tc.tile_pool(name="pp_attn", bufs=1, space="PSUM"))
        pp_den = ctx.enter_context(
            tc.tile_pool(name="pp_den", bufs=1, space="PSUM"))
        pp_t = ctx.enter_context(tc.tile_pool(name="pp_t", bufs=2, space="PSUM"))

        # ------------- internal DRAM -------------
        # transposed activations: [MT, 128, KT*128] (contiguous lhsT rows)
        xTb = {0: dram.tile([MT, 128, EMB], BF16, name="xT0"),
               1: dram.tile([MT, 128, D1], BF16, name="xT1"),
               2: dram.tile([MT, 128, D2], BF16, name="xT2")}
        AUW = {1: D1 + 32, 2: D2 + 32, 3: D3 + 32}
        augF_l = {li: dram.tile([2, HROWS, AUW[li]], BF16,
                                name=f"augF{li}l") for li in (1, 2, 3)}
        augF_f = {li: dram.tile([NCORES, 2, HROWS, AUW[li]], BF16,
                                name=f"augF{li}f", addr_space="Shared")
                  for li in (1, 2, 3)}
        pool_in = dram.tile([G, FEAT], F32, name="pool_in")
        pool_out = dram.tile([G, FEAT], F32, name="pool_out",
                             addr_space="Shared")

        # ------------- constants -------------
        ident = cpool.tile([128, 128], BF16, name="ident")
        make_identity(nc, ident[:])

        W0_sb = wpool.tile([TILEF, EMB], F32, name="W0sb")
        nc.sync.dma_start(W0_sb[:], W0_in[:, :])
        b0_sb = wpool.tile([128, EMB], BF16, name="b0sb")
        nc.sync.dma_start(b0_sb[:], b0_in[:, :].to_broadcast([128, EMB]))
        bf_sb = wpool.tile([G, FEAT], F32, name="bfsb")
        nc.sync.dma_start(bf_sb[:], bf_in[:, :].to_broadcast([G, FEAT]))
        crecip_sb = wpool.tile([G, 1], F32, name="crecipsb")
        nc.sync.dma_start(crecip_sb[:], crecip_in[:, :])
        gone_sb = wpool.tile([128, MT, G], BF16, name="gonesb")
        nc.sync.dma_start(
            gone_sb[:], gone_in[:, :].rearrange("(b p) g -> p b g", p=128))
        meta_sb = cpool.tile([128, TOT], I32, name="metasb")
        nc.sync.dma_start(meta_sb[:], meta_in[:, :])
        oh_sb = cpool.tile([128, TOT, 128], BF16, name="ohsb")
        nc.sync.dma_start(oh_sb[:], oh_in[:, :, :])

        def load_layer_weights(nm, kdim, ndim):
            ts_ = []
            for k in range(kdim // 128):
                t = wpool.tile([128, ndim], BF16, name=f"{nm}k{k}",
                               tag=f"Wk{k}", bufs=1)
                nc.sync.dma_start(t[:], Ws_in[nm][k * 128:(k + 1) * 128, :])
                ts_.append(t)
            return ts_

        def leaky(out_ap, in_ap, tmp_tile):
            nc.vector.tensor_scalar_mul(tmp_tile, in_ap, SLOPE)
            nc.vector.tensor_tensor(out=out_ap, in0=in_ap, in1=tmp_tile,
                                    op=OP.max)

        def transpose_out(xn, dd, dst_row):
            """PE-transpose xn [128, dd], packing 8 k-tiles per PSUM bank,
            then one copy + one DMA per pack into dst_row [128, dd]."""
            KTo = dd // 128
            for j0 in range(0, KTo, 8):
                npk = min(8, KTo - j0)
                xtp = pp_t.tile([128, 8, 128], BF16, name="xtp", tag="tps")
                for j in range(npk):
                    k = j0 + j
                    nc.tensor.transpose(xtp[:, j, :],
                                        xn[:, k * 128:(k + 1) * 128], ident[:])
                xts = sb.tile([128, 8, 128], BF16, name="xts", tag="xts",
                              bufs=2)
                nc.vector.tensor_copy(xts[:, 0:npk, :], xtp[:, 0:npk, :])
                nc.sync.dma_start(
                    dst_row[:, j0 * 128:(j0 + npk) * 128],
                    xts[:, 0:npk, :].rearrange("p a b -> p (a b)"))

        # =========== layer 0: x0 = leaky(x @ W0 + b0), stored transposed ====
        for m in range(MT):
            ms = slice(m * 128, (m + 1) * 128)
            lhs0 = sb.tile([TILEF, 128], F32, name="lhs0", tag="lhs0", bufs=2)
            nc.sync.dma_start(lhs0[:], xT_in[:, ms])
            ps = pp_dense.tile([128, 512], F32, name="ps0", tag="dps")
            nc.tensor.matmul(ps[:, :EMB], lhsT=lhs0[:], rhs=W0_sb[:],
                             start=True, stop=True)
            hraw = sb.tile([128, EMB], BF16, name="hraw0", tag="x0t", bufs=2)
            nc.vector.tensor_tensor(out=hraw[:], in0=ps[:, :EMB],
                                    in1=b0_sb[:], op=OP.add)
            tmp = sb.tile([128, EMB], BF16, name="tmp0", tag="x0tmp", bufs=2)
            xo = sb.tile([128, EMB], BF16, name="xo0", tag="x0o", bufs=2)
            leaky(xo[:], hraw[:], tmp[:])
            transpose_out(xo[:], EMB, xTb[0][m])

        # =========== GAT layers ===========
        def gat_layer(li, d_in, dd, heads, xT_prev, xT_out):
            ch = dd // heads
            KT = d_in // 128
            NCH = dd // 512
            W = load_layer_weights(f"W{li}", d_in, dd + 16)

            # ---- dense stage ----
            for m in range(MT):
                rows = min(128, NLOC - m * 128)
                if rows <= 0:
                    continue
                half = m * 128 // HROWS
                r0 = m * 128 - half * HROWS
                lhs_all = sb.tile([128, d_in], BF16, name=f"lhsT{li}",
                                  tag="lhsT", bufs=2)
                nc.sync.dma_start(lhs_all[:], xT_prev[m])
                h_sb = sb.tile([128, dd + 16], BF16, name=f"h{li}",
                               tag="h_sb", bufs=2)
                for nchunk in range(NCH):
                    nsl = slice(nchunk * 512, (nchunk + 1) * 512)
                    ps = pp_dense.tile([128, 512], F32, name=f"dps{li}",
                                       tag="dps")
                    for k in range(KT):
                        nc.tensor.matmul(
                            ps[:], lhsT=lhs_all[:, k * 128:(k + 1) * 128],
                            rhs=W[k][:, nsl],
                            start=(k == 0), stop=(k == KT - 1))
                    nc.vector.tensor_copy(h_sb[:, nsl], ps[:])
                tail_ps = pp_t.tile([128, 16], F32, name=f"tlps{li}",
                                    tag="tps")
                for k in range(KT):
                    nc.tensor.matmul(
                        tail_ps[:], lhsT=lhs_all[:, k * 128:(k + 1) * 128],
                        rhs=W[k][:, dd:dd + 16],
                        start=(k == 0), stop=(k == KT - 1))
                nc.vector.tensor_copy(h_sb[:, dd:dd + 16], tail_ps[:])
                nc.sync.dma_start(augF_l[li][half, r0:r0 + rows, 0:dd + 16],
                                  h_sb[:rows, :])

            nc.gpsimd.collective_compute(
                "AllGather", OP.bypass,
                replica_groups=[list(range(NCORES))],
                ins=[augF_l[li][:, :, :].opt()],
                outs=[augF_f[li][:, :, :, :].opt()])
            aug_rows = augF_f[li][:, :, :, :].rearrange("c a r w -> (c a r) w")

            w_allf = sb.tile([128, TOT, 8], F32, name=f"wallf{li}",
                             tag="wallf", bufs=2)
            pool_ps = None
            if li == 3:
                pool_ps = pp_dense.tile([G, 512], F32, name="poolps",
                                        tag="dps")

            for b in range(NB):
                rows = sizes[b]
                r0 = b * 128
                half = r0 // HROWS
                hr0 = r0 - half * HROWS
                o = offs[b]
                TB = btiles[b]
                # ---- pass A: tail gathers -> w, den, 1/den ----
                ed_blk = sb.tile([128, 8], BF16, name=f"edblk{li}",
                                 tag="edblk", bufs=3)
                if rows < 128:
                    nc.vector.memset(ed_blk[:], 0.0)
                nc.sync.dma_start(
                    ed_blk[:rows, :],
                    augF_l[li][half, hr0:hr0 + rows, dd + 8:dd + 16])
                oht_sl = slabp.tile([128, TMAX, 128], BF16, name=f"oht{li}",
                                    tag="ohtsl")
                nc.sync.dma_start(oht_sl[:, 0:TB, :],
                                  oht_in[:, o:o + TB, :])
                edpe_ps = pp_t.tile([128, TMAX, 8], F32, name=f"edpe{li}",
                                    tag="tps")
                den_ps = pp_den.tile([128, 8], F32, name=f"den{li}", tag="den")
                w_all = sb.tile([128, TMAX, 8], BF16, name=f"wall{li}",
                                tag="wall", bufs=2)
                # narrow layer: one full-row gather per tile, held across
                # both passes (halves the gpsimd descriptor-gen load)
                hold = dd <= 512
                if hold:
                    gfh = gfp.tile([128, TMAX, dd + 32], BF16,
                                   name=f"gfh{li}", tag="gfh", bufs=2)
                for t in range(TB):
                    ti = o + t
                    if hold:
                        gs = gfh[:, t, dd:dd + 32]
                        nc.gpsimd.indirect_dma_start(
                            out=gfh[:, t, :], out_offset=None,
                            in_=aug_rows,
                            in_offset=IndirectOffsetOnAxis(
                                ap=meta_sb[:, ti:ti + 1], axis=0))
                    else:
                        gst = gsp.tile([128, 32], BF16, name=f"gs{li}",
                                       tag="gs", bufs=4)
                        gs = gst[:]
                        nc.gpsimd.indirect_dma_start(
                            out=gst[:], out_offset=None,
                            in_=aug_rows, element_offset=dd,
                            in_offset=IndirectOffsetOnAxis(
                                ap=meta_sb[:, ti:ti + 1], axis=0))
                    nc.tensor.matmul(edpe_ps[:, t, :], lhsT=oht_sl[:, t, :],
                                     rhs=ed_blk[:, 0:8], start=True, stop=True)
                    e_t = sb.tile([128, 8], F32, name=f"et{li}", tag="et",
                                  bufs=3)
                    nc.vector.tensor_tensor(out=e_t[:], in0=gs[:, 0:8],
                                            in1=edpe_ps[:, t, :], op=OP.add)
                    w1 = sb.tile([128, 8], F32, name=f"w1{li}", tag="w1",
                                 bufs=3)
                    w2 = sb.tile([128, 8], F32, name=f"w2{li}", tag="w2",
                                 bufs=3)
                    nc.scalar.activation(w1[:], e_t[:], ACT_EXP)
                    nc.scalar.activation(w2[:], e_t[:], ACT_EXP, scale=SLOPE)
                    nc.vector.tensor_tensor(out=w_allf[:, ti, :], in0=w1[:],
                                            in1=w2[:], op=OP.max)
                    nc.vector.tensor_copy(w_all[:, t, :], w_allf[:, ti, :])
                    nc.tensor.matmul(den_ps[:], lhsT=oh_sb[:, ti, :],
                                     rhs=w_all[:, t, :],
                                     start=(t == 0), stop=(t == TB - 1))
                den_sb = sb.tile([128, 8], F32, name=f"densb{li}",
                                 tag="densb", bufs=2)
                nc.vector.tensor_scalar_add(den_sb[:], den_ps[:], 1e-16)
                rd = sb.tile([128, 8], F32, name=f"rd{li}", tag="rd", bufs=2)
                nc.vector.reciprocal(rd[:], den_sb[:])

                # ---- pass B: feature gathers + weighted scatter-add ----
                out_ps = pp_attn.tile([128, dd], F32, name=f"oat{li}",
                                      tag="oat")
                for t in range(TB):
                    ti = o + t
                    if hold:
                        gf = gfh[:, t, 0:dd]
                    else:
                        gft = gfp.tile([128, dd], BF16, name=f"gf{li}",
                                       tag="gf", bufs=3)
                        nc.gpsimd.indirect_dma_start(
                            out=gft[:], out_offset=None,
                            in_=aug_rows,
                            in_offset=IndirectOffsetOnAxis(
                                ap=meta_sb[:, ti:ti + 1], axis=0))
                        gf = gft[:]
                    fw = sb.tile([128, dd], BF16, name=f"fw{li}", tag="fw",
                                 bufs=2)
                    for h in range(heads):
                        hs = slice(h * ch, (h + 1) * ch)
                        if SCALAR_OFFLOAD and h % 2 == 1:
                            nc.scalar.activation(
                                fw[:, hs], gf[:, hs], ACT_COPY,
                                scale=w_allf[:, ti, h:h + 1])
                        else:
                            nc.vector.tensor_scalar_mul(
                                fw[:, hs], gf[:, hs], w_allf[:, ti, h:h + 1])
                    for nchunk in range(NCH):
                        nsl = slice(nchunk * 512, (nchunk + 1) * 512)
                        nc.tensor.matmul(out_ps[:, nsl],
                                         lhsT=oh_sb[:, ti, :],
                                         rhs=fw[:, nsl],
                                         start=(t == 0), stop=(t == TB - 1))
                # epilogue: xn = leaky(out * 1/den) per head (biases are 0)
                xc = sb.tile([128, dd], BF16, name=f"xc{li}", tag="xc",
                             bufs=2)
                for h in range(heads):
                    hs = slice(h * ch, (h + 1) * ch)
                    if SCALAR_OFFLOAD and h % 2 == 1:
                        nc.scalar.activation(xc[:, hs], out_ps[:, hs],
                                             ACT_COPY, scale=rd[:, h:h + 1])
                    else:
                        nc.vector.tensor_scalar_mul(xc[:, hs], out_ps[:, hs],
                                                    rd[:, h:h + 1])
                xt_ = sb.tile([128, dd], BF16, name=f"xt{li}", tag="xtm",
                              bufs=1)
                xn = sb.tile([128, dd], BF16, name=f"xn{li}", tag="xn",
                             bufs=2)
                leaky(xn[:], xc[:], xt_[:])
                if li < 3:
                    transpose_out(xn[:], dd, xT_out[b])
                else:
                    nc.tensor.matmul(pool_ps[:], lhsT=gone_sb[:, b, :],
                                     rhs=xn[:, :FEAT], start=(b == 0),
                                     stop=(b == NB - 1))
                    if b == NB - 1:
                        psum_sb = sb.tile([G, FEAT], F32, name="psum_sb",
                                          tag="fc16", bufs=3)
                        nc.vector.tensor_copy(psum_sb[:], pool_ps[:])
                        nc.sync.dma_start(pool_in[:, :], psum_sb[:])

        gat_layer(1, EMB, D1, HEADS, xTb[0], xTb[1])
        gat_layer(2, D1, D2, HEADS, xTb[1], xTb[2])
        gat_layer(3, D2, D3, 1, xTb[2], None)

        # =========== pooling reduce + FC ===========
        nc.gpsimd.collective_compute(
            "AllReduce", OP.add, replica_groups=[list(range(NCORES))],
            ins=[pool_in[:, :].opt()], outs=[pool_out[:, :].opt()])
        psum_all = sb.tile([G, FEAT], F32, name="psum_all", tag="fc16", bufs=3)
        nc.sync.dma_start(psum_all[:], pool_out[:, :])
        mean_f = sb.tile([G, FEAT], F32, name="mean_f", tag="fc16", bufs=3)
        nc.vector.tensor_scalar_mul(mean_f[:], psum_all[:], crecip_sb[:, 0:1])
        mean_bf = sb.tile([G, FEAT], BF16, name="mean_bf", tag="fc16", bufs=3)
        nc.vector.tensor_copy(mean_bf[:], mean_f[:])
        Wf_sb = load_layer_weights("Wf", OUT, FEAT)
        fc_ps = pp_dense.tile([G, 512], F32, name="fcps", tag="dps")
        for k in range(OUT // 128):
            mT_ps = pp_t.tile([128, G], BF16, name="mTps", tag="tps")
            nc.tensor.transpose(mT_ps[:], mean_bf[:, k * 128:(k + 1) * 128],
                                ident[:G, :G])
            mT = sb.tile([128, G], BF16, name="mT", tag="mT", bufs=2)
            nc.vector.tensor_copy(mT[:], mT_ps[:])
            nc.tensor.matmul(fc_ps[:], lhsT=mT[:], rhs=Wf_sb[k][:],
                             start=(k == 0), stop=(k == OUT // 128 - 1))
        fc_raw = sb.tile([G, FEAT], F32, name="fc_raw", tag="fc16", bufs=3)
        nc.vector.tensor_tensor(out=fc_raw[:], in0=fc_ps[:], in1=bf_sb[:],
                                op=OP.add)
        fc_t = sb.tile([G, FEAT], F32, name="fc_t", tag="fc16", bufs=3)
        fc_o = sb.tile([G, FEAT], F32, name="fc_o", tag="fc16", bufs=3)
        leaky(fc_o[:], fc_raw[:], fc_t[:])
        nc.sync.dma_start(out_ext[:, :], fc_o[:])

    nc.finalize()
    return nc


# --------------------------------------------------------------------------
# Entry point
# --------------------------------------------------------------------------

def make_cfg(N):
    NLOC = N // NCORES
    NB = (NLOC + 127) // 128
    sizes = [128] * (NLOC // 128) + ([NLOC % 128] if NLOC % 128 else [])
    NROWS = NB * 128
    HROWS = (NB // 2) * 128
    return {"N": N, "NLOC": NLOC, "NB": NB, "sizes": sizes, "NROWS": NROWS,
            "HROWS": HROWS}


def prepare_in_maps(inputs, cfg=None):
    x = np.asarray(inputs["x"], dtype=np.float32)
    N = x.shape[0]
    if cfg is None:
        cfg = make_cfg(N)
    cfg, metas, ohs, ohts, gone, xT, crecip = preprocess(
        x, inputs["edge_index"], inputs["batch"], cfg)

    def b16(a):
        return np.asarray(a, dtype=np.float32).astype(bf16np)

    # biases are all zero in this model; assert so the kernel can skip them
    for bn in ("b1", "b2", "b3"):
        assert np.abs(np.asarray(inputs[bn])).max() == 0.0

    shared = {
        "W0": np.asarray(inputs["W0"], np.float32),
        "b0": b16(inputs["b0"]).reshape(1, -1),
        "W1": _fold_aug_weights(inputs["W1"], inputs["a1s"], inputs["a1d"],
                                8, 256),
        "W2": _fold_aug_weights(inputs["W2"], inputs["a2s"], inputs["a2d"],
                                8, 256),
        "W3": _fold_aug_weights(inputs["W3"], inputs["a3s"], inputs["a3d"],
                                1, 512),
        "Wf": b16(inputs["Wf"]),
        "bfc": np.asarray(inputs["bf"], np.float32).reshape(1, -1),
        "crecip": crecip,
    }
    in_maps = []
    for c in range(NCORES):
        m = dict(shared)
        m["xT_in"] = xT[c]
        m["meta"] = metas[c]
        m["oh"] = ohs[c]
        m["oht"] = ohts[c]
        m["gone"] = gone[c]
        in_maps.append(m)
    return cfg, in_maps


_CACHE = {}


def _ensure_ntff_hook():
    """Register the axon NTFF profiling hook if the antenv shim is missing."""
    import sys
    import types
    try:
        from antenv.axon_hooks import get_axon_ntff_profile_hook  # noqa: F401
        return
    except ImportError:
        pass
    try:
        import antenv
        from trn_agent_boot.trn_boot import _ntff_profile_via_ctypes
    except ImportError:
        return
    mod = types.ModuleType("antenv.axon_hooks")
    mod._hook = None
    mod.set_axon_ntff_profile_hook = lambda h: setattr(mod, "_hook", h)
    mod.get_axon_ntff_profile_hook = lambda: mod._hook
    sys.modules["antenv.axon_hooks"] = mod
    antenv.axon_hooks = mod
    try:
        mod._hook = _ntff_profile_via_ctypes("/opt/axon/libaxon_pjrt.so")
    except Exception:
        mod._hook = None


def kernel(**inputs) -> np.ndarray:
    from concourse.bass_utils import run_bass_kernel_spmd
    if os.environ.get("GNN_TRACE"):
        _ensure_ntff_hook()
    cfg, in_maps = prepare_in_maps(inputs)
    key = (cfg["N"], cfg["NB"], tuple(cfg["tiles"]))
    if key not in _CACHE:
        _CACHE[key] = build_program(cfg)
    nc = _CACHE[key]
    res = run_bass_kernel_spmd(nc, in_maps, core_ids=list(range(NCORES)),
                               trace=bool(os.environ.get("GNN_TRACE")))
    out = res.results[0]["out"]
    kernel.last_exec_time_ns = res.exec_time_ns
    kernel.last_results = res
    return np.asarray(out, dtype=np.float32)
